# revision 28
# baseline (speedup 1.0000x reference)
"""Trainium2 Bass kernel for multi-lengthscale RBF kernel self-attention.

Reference computation (B=2, N=4096, D=128, 4 heads of 32):
  d2[b,i,j] = ||coords[b,i]-coords[b,j]||^2
  att_h = softmax-ish: exp(-d2/ls_h^2) row-normalized (+1e-8), ls = [0.5,1,2,4]
  out = concat_h(att_h @ (features @ Wv[h] + bv[h])) @ Wo + bo

Device strategy (8 cores, query rows sharded):
  * Gram trick: -d2[j,i] = 2 xj.xi - |xj|^2 - |xi|^2 computed as ONE K=5
    matmul per (batch, j-block): lhsT rows [x,y,z,-|x|^2,1] (all j),
    rhs rows [2x,2y,2z,1,-|x|^2] (this core's 512 queries).
  * e1=exp(G/16) [ls=4], e2=exp(G/4) [ls=2], e3=exp(G) [ls=1] on ACT,
    e4=(e3^2)^2 [ls=0.5] on DVE.  All <= 1, no overflow.
  * att_h @ V_h with V_h (+ ones column for rowsums) as the 33-col
    stationary operand; streams each w-tile through the PE once.
    PSUM [33, 512*4]: rows 0..31 = head outputs^T, row 32 = rowsums.
  * Normalize: rowsums -> SBUF (DMA partition-scatter), reciprocal,
    broadcast down 128 partitions via indicator matmul, multiply.
  * Wo projection on PE; output stored [o, i] (transposed), host fixes.

Host does only O(N*D) marshalling: coords augmentation, V = F@Wv (+ones),
bo_eff = bo + bv@Wo added at the end, final transpose.
"""

import numpy as np

B = 2
N = 4096
NCORES = 8
NQ = N // NCORES          # 512 query rows per core per batch
P = 128                   # partitions / j-block size
NJB = N // P              # 32 j-blocks
VW = 33                   # V columns per head incl. ones column
VROW = 4 * VW             # 132 cols per j-block in vall
D = 128
KG = 13                   # Gram K rows (bf16 hi/lo split, see _prep)

_BUILT = {}


def _build():
    import concourse.bass as bass
    import concourse.bacc as bacc
    import concourse.mybir as mybir
    import concourse.tile as tile

    f32 = mybir.dt.float32
    f32r = mybir.dt.float32r
    bf16 = mybir.dt.bfloat16
    AF = mybir.ActivationFunctionType

    nc = bacc.Bacc("TRN2", target_bir_lowering=False, debug=False,
                   enable_asserts=True, num_devices=NCORES)

    grama = nc.dram_tensor("grama", (B, KG, N), bf16, kind="ExternalInput").ap()
    gramr = nc.dram_tensor("gramr", (B, KG, NQ), bf16, kind="ExternalInput").ap()
    vall_d = nc.dram_tensor("vall", (B, P, NJB * VROW), bf16, kind="ExternalInput").ap()
    outm = nc.dram_tensor("outm", (B, VW, 4 * NQ), f32, kind="ExternalOutput").ap()

    with tile.TileContext(nc) as tc:
        with (
            tc.tile_pool(name="const", bufs=1) as cp,
            tc.tile_pool(name="elem", bufs=3) as ep,
            tc.tile_pool(name="epil", bufs=2) as lp,
            tc.tile_pool(name="gps", bufs=2, space="PSUM") as gp,
            tc.tile_pool(name="aps", bufs=1, space="PSUM") as ap_,
        ):
            ga = {}
            gr = {}
            va = {}
            for b in range(B):
                ga[b] = cp.tile([KG, N], bf16, tag=f"ga{b}", name=f"ga{b}")
                nc.sync.dma_start(ga[b][:], grama[b])
                gr[b] = cp.tile([KG, NQ], bf16, tag=f"gr{b}", name=f"gr{b}")
                nc.sync.dma_start(gr[b][:], gramr[b])
                va[b] = cp.tile([P, NJB * VROW], bf16, tag=f"va{b}", name=f"va{b}")
                # split the 2.1MB load across DMA queues
                nch = 8
                w = NJB * VROW // nch
                for c in range(nch):
                    nc.sync.dma_start(va[b][:, c * w:(c + 1) * w],
                                      vall_d[b][:, c * w:(c + 1) * w])

            GW = 2 * NQ           # pair width: 2 j-blocks per G tile
            for b in range(B):
                # ---- main loop: attention over all j-blocks ----
                att = ap_.tile([VW, 4 * NQ], f32, tag="att")
                for jg in range(NJB // 2):
                    g = gp.tile([P, GW], f32, tag="g")
                    for k in range(2):
                        jb = 2 * jg + k
                        nc.tensor.matmul(g[:, NQ * k:NQ * (k + 1)],
                                         ga[b][:, P * jb:P * (jb + 1)],
                                         gr[b][:], start=True, stop=True)
                    # seeds on ACT (bf16 out), 4th-power chains on DVE
                    # (2x mode), all at pair width to amortize overheads
                    e3 = ep.tile([P, GW], bf16, tag="e3")
                    nc.scalar.activation(e3[:], g[:], AF.Exp, scale=1.0)
                    e1 = ep.tile([P, GW], bf16, tag="e1")
                    nc.scalar.activation(e1[:], g[:], AF.Exp, scale=1.0 / 16.0)
                    e4a = ep.tile([P, GW], bf16, tag="e4a")
                    nc.vector.tensor_mul(e4a[:], e3[:], e3[:])
                    e4 = ep.tile([P, GW], bf16, tag="e4")
                    nc.vector.tensor_mul(e4[:], e4a[:], e4a[:])
                    e2a = ep.tile([P, GW], bf16, tag="e2a")
                    nc.vector.tensor_mul(e2a[:], e1[:], e1[:])
                    e2 = ep.tile([P, GW], bf16, tag="e2")
                    nc.vector.tensor_mul(e2[:], e2a[:], e2a[:])
                    wmap = {0: e4, 1: e3, 2: e2, 3: e1}
                    for k in range(2):
                        jb = 2 * jg + k
                        for h in (1, 3, 0, 2):
                            nc.tensor.matmul(
                                att[:, NQ * h:NQ * (h + 1)],
                                va[b][:, VROW * jb + VW * h:
                                      VROW * jb + VW * h + VW],
                                wmap[h][:, NQ * k:NQ * (k + 1)],
                                start=(jb == 0), stop=(jb == NJB - 1))

                # ---- epilogue: spill raw numerators+rowsums; host
                # normalizes and applies Wo (O(N*D^2) marshalling, same
                # class as the host-side V projection). The single copy
                # frees the att PSUM banks fast so batch b+1 can start
                # accumulating.
                attc = lp.tile([VW, 4 * NQ], f32, tag="attc")
                nc.scalar.copy(attc[:], att[:])
                nc.sync.dma_start(outm[b], attc[:])

    nc.compile()
    return nc


def _prep(features, coords, Wv, bv, Wo, bo):
    import ml_dtypes
    bf = ml_dtypes.bfloat16

    coords = np.asarray(coords, np.float32)
    features = np.asarray(features, np.float32)
    Wv = np.asarray(Wv, np.float32)
    bv = np.asarray(bv, np.float32)
    Wo = np.asarray(Wo, np.float32)
    bo = np.asarray(bo, np.float32)

    # bf16 hi/lo split so the K=13 bf16 Gram matmul carries ~16-bit
    # mantissa: G[j,i] = 2 xj.xi - |xj|^2 - |xi|^2 with
    # 2 xj.xi ~ 2(xjh.xih + xjh.xil + xjl.xih)  (lo*lo dropped)
    xh = coords.astype(bf).astype(np.float32)        # [B, N, 3]
    xl = coords - xh
    sq = (coords ** 2).sum(-1)                       # [B, N]
    sqh = sq.astype(bf).astype(np.float32)
    sql = sq - sqh
    one = np.ones_like(sq)
    za = [xh[..., 0], xh[..., 1], xh[..., 2],        # pair w/ 2*xih
          xh[..., 0], xh[..., 1], xh[..., 2],        # pair w/ 2*xil
          xl[..., 0], xl[..., 1], xl[..., 2],        # pair w/ 2*xih
          -sqh, -sql, one, one]
    zr = [2 * xh[..., 0], 2 * xh[..., 1], 2 * xh[..., 2],
          2 * xl[..., 0], 2 * xl[..., 1], 2 * xl[..., 2],
          2 * xh[..., 0], 2 * xh[..., 1], 2 * xh[..., 2],
          one, one, -sqh, -sql]
    grama = np.stack(za, axis=1).astype(bf)          # [B, 13, N]
    gramr = np.stack(zr, axis=1).astype(bf)          # [B, 13, N]

    # V (no bv: folded into bo_eff) with ones column per head, laid out
    # [B, 128, NJB*132]: col jb*132 + h*33 + k = V[b, jb*128+p, h, k]
    v = np.einsum('bnd,hdk->bnhk', features, Wv)     # [B, N, 4, 32]
    vaug = np.concatenate([v, np.ones((B, N, 4, 1), np.float32)], axis=-1)
    # [B, N, 4, 33] -> [B, NJB, P, 132] -> [B, P, NJB, 132]
    vall = vaug.reshape(B, NJB, P, VROW).transpose(0, 2, 1, 3).reshape(
        B, P, NJB * VROW)
    vall = np.ascontiguousarray(vall).astype(bf)

    bo_eff = bo + bv.reshape(-1) @ Wo                # [128]
    return grama, gramr, vall, Wo, bo_eff


def kernel(features, coords, Wv, bv, Wo, bo):
    from concourse import bass_utils

    grama, gramr, vall, wo, bo_eff = _prep(features, coords, Wv, bv, Wo, bo)

    if "nc" not in _BUILT:
        _BUILT["nc"] = _build()
    nc = _BUILT["nc"]

    in_maps = []
    for c in range(NCORES):
        sl = slice(c * NQ, (c + 1) * NQ)
        in_maps.append({
            "grama": grama,
            "gramr": np.ascontiguousarray(gramr[:, :, sl]),
            "vall": vall,
        })
    res = bass_utils.run_bass_kernel_spmd(nc, in_maps,
                                          core_ids=list(range(NCORES)),
                                          trace=_BUILT.get("trace", False),
                                          tmpdir=_BUILT.get("tmpdir"))
    _BUILT["last_results"] = res

    # outm[b, k, h*NQ+i]: rows 0..31 are head-h numerators^T for this
    # core's queries, row 32 the rowsums. Normalize + Wo on host.
    mh = np.empty((B, N, D), np.float32)
    for c in range(NCORES):
        om = res.results[c]["outm"]                  # [B, 33, 4*NQ]
        m = om[:, :32, :].reshape(B, 32, 4, NQ)      # [b, k, h, i]
        r = om[:, 32, :].reshape(B, 1, 4, NQ)
        mn = (m / r).transpose(0, 3, 2, 1)           # [b, i, h, k]
        mh[:, c * NQ:(c + 1) * NQ, :] = mn.reshape(B, NQ, D)
    out = mh @ wo + bo_eff[None, None, :]
    return out



# revision 31
# speedup vs baseline: 1.1656x; 1.1656x over previous
"""Trainium2 Bass kernel for multi-lengthscale RBF kernel self-attention.

Reference computation (B=2, N=4096, D=128, 4 heads of 32):
  d2[b,i,j] = ||coords[b,i]-coords[b,j]||^2
  att_h = exp(-d2/ls_h^2) row-normalized (+1e-8), ls = [0.5,1,2,4]
  out = concat_h(att_h @ (features @ Wv[h] + bv[h])) @ Wo + bo

Device strategy (8 cores, query rows sharded):
  * Gram trick: G[j,i] = -d2 = 2 xj.xi - |xj|^2 - |xi|^2 as ONE K=13
    bf16 matmul per (batch, j-block) using a hi/lo split of coords and
    norms (~16-bit mantissa; lo*lo term dropped).
  * ls=4 head via exact degree-7 polynomial features of the 3D coords:
    exp(-d2/16) = a_i a_j e^(xj.xi/8), e^u fit by Chebyshev deg 7 on
    the data's actual u-range -> 120 monomial features phi/psi. Head
    output = (psi a V4)^T-projected M (host, O(N*D^2)) streamed as TWO
    K=120 matmuls (bf16 hi/lo of M) per batch. Removes one exp AND one
    att stream per j-block.
  * Remaining weights: e2 = exp(G/4) on ACT every block; e3 = exp(G)
    alternates: even blocks on ACT, odd blocks as (e2^2)^2 on DVE;
    e4 = (e3^2)^2 on DVE. Balances ACT (~1.5 exp/blk) vs DVE (~3 mul/blk),
    all bf16 (DVE 2x mode).
  * att_h @ V_h with bf16 V_h (+ ones column for rowsums) as the 33-col
    stationary operand; bf16 weight streams at 1 cyc/row.
    PSUM att [33, 512*4]: rows 0..31 head numerators^T, row 32 rowsums.
  * Epilogue: single ACT copy PSUM->SBUF, DMA out raw numerators +
    rowsums; host normalizes and applies Wo (same O(N*D^2) marshalling
    class as the host-side V projection).
"""

import numpy as np

B = 2
N = 4096
NCORES = 8
NQ = N // NCORES          # 512 query rows per core per batch
P = 128                   # partitions / j-block size
NJB = N // P              # 32 j-blocks
VW = 33                   # V columns per head incl. ones column
NH = 3                    # heads streamed on device (ls=0.5, 1, 2)
VROW = NH * VW            # 99 cols per j-block in vall
D = 128
KG = 13                   # Gram K rows (bf16 hi/lo split, see _prep)
NF = 120                  # polynomial features for the ls=4 head

_BUILT = {}


def _build():
    import concourse.bass as bass
    import concourse.bacc as bacc
    import concourse.mybir as mybir
    import concourse.tile as tile

    f32 = mybir.dt.float32
    bf16 = mybir.dt.bfloat16
    AF = mybir.ActivationFunctionType

    nc = bacc.Bacc("TRN2", target_bir_lowering=False, debug=False,
                   enable_asserts=True, num_devices=NCORES)

    grama = nc.dram_tensor("grama", (B, KG, N), bf16, kind="ExternalInput").ap()
    gramr = nc.dram_tensor("gramr", (B, KG, NQ), bf16, kind="ExternalInput").ap()
    vall_d = nc.dram_tensor("vall", (B, P, NJB * VROW), bf16, kind="ExternalInput").ap()
    phia_d = nc.dram_tensor("phia", (B, NF, NQ), bf16, kind="ExternalInput").ap()
    m4_d = nc.dram_tensor("m4", (B, 2, NF, VW), bf16, kind="ExternalInput").ap()
    outm = nc.dram_tensor("outm", (B, VW, 4 * NQ), f32, kind="ExternalOutput").ap()

    with tile.TileContext(nc) as tc:
        with (
            tc.tile_pool(name="const", bufs=1) as cp,
            tc.tile_pool(name="elem", bufs=3) as ep,
            tc.tile_pool(name="epil", bufs=2) as lp,
            tc.tile_pool(name="gps", bufs=4, space="PSUM") as gp,
            tc.tile_pool(name="aps", bufs=1, space="PSUM") as ap_,
        ):
            ga = {}
            gr = {}
            va = {}
            ph = {}
            m4 = {}
            for b in range(B):
                ga[b] = cp.tile([KG, N], bf16, tag=f"ga{b}", name=f"ga{b}")
                nc.sync.dma_start(ga[b][:], grama[b])
                gr[b] = cp.tile([KG, NQ], bf16, tag=f"gr{b}", name=f"gr{b}")
                nc.sync.dma_start(gr[b][:], gramr[b])
                ph[b] = cp.tile([NF, NQ], bf16, tag=f"ph{b}", name=f"ph{b}")
                nc.sync.dma_start(ph[b][:], phia_d[b])
                m4[b] = cp.tile([NF, 2 * VW], bf16, tag=f"m4{b}", name=f"m4{b}")
                nc.sync.dma_start(m4[b][:, 0:VW], m4_d[b][0])
                nc.sync.dma_start(m4[b][:, VW:2 * VW], m4_d[b][1])
                va[b] = cp.tile([P, NJB * VROW], bf16, tag=f"va{b}", name=f"va{b}")
                # split the 1.6MB load across DMA queues
                nch = 8
                w = NJB * VROW // nch
                for c in range(nch):
                    nc.sync.dma_start(va[b][:, c * w:(c + 1) * w],
                                      vall_d[b][:, c * w:(c + 1) * w])

            for b in range(B):
                # ---- main loop: attention over all j-blocks ----
                att = ap_.tile([VW, 4 * NQ], f32, tag="att")
                # ls=4 head: two K=120 matmuls (M hi + lo), done once
                nc.tensor.matmul(att[:, 3 * NQ:4 * NQ], m4[b][:, 0:VW],
                                 ph[b][:], start=True, stop=False)
                nc.tensor.matmul(att[:, 3 * NQ:4 * NQ], m4[b][:, VW:2 * VW],
                                 ph[b][:], start=False, stop=True)
                for jb in range(NJB):
                    g = gp.tile([P, NQ], f32, tag="g")
                    nc.tensor.matmul(g[:], ga[b][:, P * jb:P * (jb + 1)],
                                     gr[b][:], start=True, stop=True)
                    # e2 on ACT every block; e3 alternates ACT/DVE to
                    # balance the engines; e4 = (e3^2)^2 on DVE. bf16.
                    e2 = ep.tile([P, NQ], bf16, tag="e2")
                    nc.scalar.activation(e2[:], g[:], AF.Exp, scale=0.25)
                    e3 = ep.tile([P, NQ], bf16, tag="e3")
                    if jb % 2 == 0:
                        nc.scalar.activation(e3[:], g[:], AF.Exp, scale=1.0)
                    else:
                        e3a = ep.tile([P, NQ], bf16, tag="e3a")
                        nc.vector.tensor_mul(e3a[:], e2[:], e2[:])
                        nc.vector.tensor_mul(e3[:], e3a[:], e3a[:])
                    e4a = ep.tile([P, NQ], bf16, tag="e4a")
                    nc.vector.tensor_mul(e4a[:], e3[:], e3[:])
                    e4 = ep.tile([P, NQ], bf16, tag="e4")
                    nc.vector.tensor_mul(e4[:], e4a[:], e4a[:])
                    wmap = {0: e4, 1: e3, 2: e2}
                    for h in (2, 1, 0):
                        nc.tensor.matmul(
                            att[:, NQ * h:NQ * (h + 1)],
                            va[b][:, VROW * jb + VW * h:VROW * jb + VW * h + VW],
                            wmap[h][:],
                            start=(jb == 0), stop=(jb == NJB - 1))

                # ---- epilogue: spill raw numerators+rowsums; host
                # normalizes and applies Wo. The single copy frees the
                # att PSUM banks fast so batch b+1 can start.
                attc = lp.tile([VW, 4 * NQ], f32, tag="attc")
                nc.scalar.copy(attc[:], att[:])
                nc.sync.dma_start(outm[b], attc[:])

    nc.compile()
    return nc


def _prep(features, coords, Wv, bv, Wo, bo):
    import ml_dtypes
    from math import factorial
    bf = ml_dtypes.bfloat16

    coords = np.asarray(coords, np.float32)
    features = np.asarray(features, np.float32)
    Wv = np.asarray(Wv, np.float32)
    bv = np.asarray(bv, np.float32)
    Wo = np.asarray(Wo, np.float32)
    bo = np.asarray(bo, np.float32)

    # bf16 hi/lo split so the K=13 bf16 Gram matmul carries ~16-bit
    # mantissa: G[j,i] = 2 xj.xi - |xj|^2 - |xi|^2 with
    # 2 xj.xi ~ 2(xjh.xih + xjh.xil + xjl.xih)  (lo*lo dropped)
    xh = coords.astype(bf).astype(np.float32)        # [B, N, 3]
    xl = coords - xh
    sq = (coords ** 2).sum(-1)                       # [B, N]
    sqh = sq.astype(bf).astype(np.float32)
    sql = sq - sqh
    one = np.ones_like(sq)
    za = [xh[..., 0], xh[..., 1], xh[..., 2],        # pair w/ 2*xih
          xh[..., 0], xh[..., 1], xh[..., 2],        # pair w/ 2*xil
          xl[..., 0], xl[..., 1], xl[..., 2],        # pair w/ 2*xih
          -sqh, -sql, one, one]
    zr = [2 * xh[..., 0], 2 * xh[..., 1], 2 * xh[..., 2],
          2 * xl[..., 0], 2 * xl[..., 1], 2 * xl[..., 2],
          2 * xh[..., 0], 2 * xh[..., 1], 2 * xh[..., 2],
          one, one, -sqh, -sql]
    grama = np.stack(za, axis=1).astype(bf)          # [B, 13, N]
    gramr = np.stack(zr, axis=1).astype(bf)          # [B, 13, N]

    # V (no bv: folded into bo_eff) with ones column per head; only the
    # 3 streamed heads (ls=0.5,1,2) go in vall. Head 3 (ls=4) is handled
    # by polynomial features below.
    v = np.einsum('bnd,hdk->bnhk', features, Wv)     # [B, N, 4, 32]
    vaug = np.concatenate([v, np.ones((B, N, 4, 1), np.float32)], axis=-1)
    v3 = vaug[:, :, :NH, :]                          # [B, N, 3, 33]
    vall = v3.reshape(B, NJB, P, VROW).transpose(0, 2, 1, 3).reshape(
        B, P, NJB * VROW)
    vall = np.ascontiguousarray(vall).astype(bf)

    # ls=4 head: exp(-d2/16) = a_i a_j e^(t/8), t = xi.xj. Chebyshev
    # deg-7 fit of e^u on the data's u-range, factored into 120 monomial
    # features. M = (psi*a*V4)^T reduction on host; phi*a streams on PE.
    import itertools
    alphas = [a for m in range(8)
              for a in itertools.product(range(m + 1), repeat=3)
              if sum(a) == m]
    assert len(alphas) == NF
    phia = np.empty((B, NF, N), np.float32)
    m4 = np.empty((B, 2, NF, VW), np.float32)
    for b in range(B):
        x = coords[b].astype(np.float64)
        umax = float((np.linalg.norm(x, axis=1).max() ** 2) / 8.0)
        cheb = np.polynomial.chebyshev.Chebyshev.interpolate(
            np.exp, 7, domain=[-umax, umax])
        bm = cheb.convert(kind=np.polynomial.Polynomial).coef
        a_i = np.exp(-(x ** 2).sum(-1) / 16.0)       # [N]
        phi = np.empty((NF, N)); psi = np.empty((NF, N))
        for k, al in enumerate(alphas):
            m = sum(al)
            coef = (bm[m] / 8.0 ** m * factorial(m) /
                    (factorial(al[0]) * factorial(al[1]) * factorial(al[2])))
            s = np.sqrt(abs(coef))
            mono = x[:, 0] ** al[0] * x[:, 1] ** al[1] * x[:, 2] ** al[2]
            phi[k] = s * mono
            psi[k] = np.sign(coef) * s * mono
        phia[b] = (phi * a_i).astype(np.float32)
        M = (psi * a_i) @ vaug[b, :, 3, :].astype(np.float64)   # [NF, 33]
        Mh = M.astype(bf).astype(np.float64)
        m4[b, 0] = Mh.astype(np.float32)
        m4[b, 1] = (M - Mh).astype(np.float32)
    phia = phia.astype(bf)
    m4 = m4.astype(bf)

    bo_eff = bo + bv.reshape(-1) @ Wo                # [128]
    return grama, gramr, vall, phia, m4, Wo, bo_eff


def kernel(features, coords, Wv, bv, Wo, bo):
    from concourse import bass_utils

    grama, gramr, vall, phia, m4, wo, bo_eff = _prep(
        features, coords, Wv, bv, Wo, bo)

    if "nc" not in _BUILT:
        _BUILT["nc"] = _build()
    nc = _BUILT["nc"]

    in_maps = []
    for c in range(NCORES):
        sl = slice(c * NQ, (c + 1) * NQ)
        in_maps.append({
            "grama": grama,
            "gramr": np.ascontiguousarray(gramr[:, :, sl]),
            "vall": vall,
            "phia": np.ascontiguousarray(phia[:, :, sl]),
            "m4": m4,
        })
    res = bass_utils.run_bass_kernel_spmd(nc, in_maps,
                                          core_ids=list(range(NCORES)),
                                          trace=_BUILT.get("trace", False),
                                          tmpdir=_BUILT.get("tmpdir"))
    _BUILT["last_results"] = res

    # outm[b, k, h*NQ+i]: rows 0..31 are head-h numerators^T for this
    # core's queries, row 32 the rowsums. Normalize + Wo on host.
    mh = np.empty((B, N, D), np.float32)
    for c in range(NCORES):
        om = res.results[c]["outm"]                  # [B, 33, 4*NQ]
        m = om[:, :32, :].reshape(B, 32, 4, NQ)      # [b, k, h, i]
        r = om[:, 32, :].reshape(B, 1, 4, NQ)
        mn = (m / r).transpose(0, 3, 2, 1)           # [b, i, h, k]
        mh[:, c * NQ:(c + 1) * NQ, :] = mn.reshape(B, NQ, D)
    out = mh @ wo + bo_eff[None, None, :]
    return out
